# revision 16
# baseline (speedup 1.0000x reference)
"""AdaptiveECE on 8 Trainium2 NeuronCores — v2 (PE-offloaded softmax sums).

Data-parallel over N=1,000,000 rows: each core streams its 125,000-row shard
of logits [N,128] through SBUF once (64MB/core, ~179us at the 358GB/s/core
HBM roofline) and reduces it to two per-row scalars:

  - mt[r] = max_c x[r,c]       exact f32 (VectorE segmented reduce_max, the
                               only 1x-rate DVE pass we keep: ~131us)
  - s[r]  = sum_c exp(x[r,c])  via TensorE+ScalarE instead of DVE:
      1. PE transposes each [128 rows, 128 cols] f32 block into PSUM
         (is_transpose matmul vs identity, 2 cyc/row: ~105us)
      2. ScalarE computes exp on the PSUM-resident transposed block, writing
         bf16 to SBUF (1 elem/cycle/lane @1.2GHz, dtype-free: ~123us)
      3. PE contracts the transposed exp over partitions (=columns) with a
         sliding one-hot stationary so each 512-row block's sums land on its
         own PSUM partition; 128 blocks accumulate into one PSUM bank
         (bf16 matmul 1 cyc/row: ~53us), then one cheap DVE copy evacuates
         65,536 row-sums at once.

  v1 ran both segmented reductions on DVE at its 1x tensor_reduce rate plus
  many small ScalarE accum ops (~226us busy on each), so compute — not the
  64MB stream — was the critical path (249us). v2 puts every engine below
  the DMA roofline: DVE ~135us, ACT ~125us, PE ~160us, DMA ~181us.

The host finishes with O(N) work as the sharding hint prescribes ("finish
ECE on one host"): conf = exp(mt)/s, accuracy = (logits[r, labels[r]] ==
mt[r]) — exact since mt is the bit-exact max — then the global sort,
equal-count bin edges, per-bin (count, conf_sum, acc_sum), and the ECE.

Layout: each partition line holds G=8 consecutive rows (4KB contiguous DMA
runs). mt column (t*G + j), partition p  <->  shard row t*G*128 + p*G + j.
Sums come out block-indexed: s_d[k, S*512 + m] = sum of row g*1024 + p*8 +
(4h + m//128) with p = m%128, where B = S*128 + k = 2g + h.
"""

import sys
import types
from contextlib import ExitStack

import numpy as np

import concourse.bass as bass
import concourse.tile as tile
from concourse import bacc, mybir
from concourse.bass_utils import run_bass_kernel_spmd
from concourse.masks import make_identity


def _ensure_ntff_hook():
    """bass_utils imports antenv.axon_hooks when tracing is requested; the
    agent image lacks that module. Recreate it (wired to the axon .so) so a
    stray BASS_TRACE=1 in the environment cannot crash the run."""
    try:
        import antenv.axon_hooks  # noqa: F401
        return
    except ImportError:
        pass
    try:
        import antenv
        import trn_agent_boot.trn_boot as tb

        mod = types.ModuleType("antenv.axon_hooks")
        holder = [None]
        mod.set_axon_ntff_profile_hook = lambda h: holder.__setitem__(0, h)
        mod.get_axon_ntff_profile_hook = lambda: holder[0]
        sys.modules["antenv.axon_hooks"] = mod
        antenv.axon_hooks = mod
        try:
            mod.set_axon_ntff_profile_hook(
                tb._ntff_profile_via_ctypes("/opt/axon/libaxon_pjrt.so")
            )
        except Exception:
            pass
    except Exception:
        pass


_ensure_ntff_hook()

N = 1_000_000
C = 128
NBINS = 15
NCORES = 8
ROWS = N // NCORES  # 125_000 per core
G = 8  # rows per partition line (4KB contiguous DMA runs)
GR = G * 128  # rows per t-group
TFULL = ROWS // GR  # 122 full t-groups
TAIL = ROWS - TFULL * GR  # 72 leftover rows
TAIL_P = TAIL // G  # 9 tail partitions
NT = TFULL + 1  # t-groups incl. zero-padded tail
NBLK = 2 * NT  # 512-row sum blocks
NSG = (NBLK + 127) // 128  # PSUM sum groups (2)
CHUNK_T = 8  # t-groups per DMA chunk
M_DELAY = 2  # t-groups between exp and its sum-matmuls (keeps PE unstalled)
USE_FP32R = False  # fp32r transposes: 1.5 cyc/row vs fp32's 2 on the PE

_CACHE: dict = {}
LAST_RESULT = None  # BassKernelResults of the most recent device run


def _build(rows: int, chunk_t: int = CHUNK_T):
    tfull = rows // GR
    tail = rows - tfull * GR
    tail_p = tail // G
    assert tail % G == 0, (rows, tail)
    nt = tfull + (1 if tail else 0)
    tt = nt * G  # mt output columns
    nblk = 2 * nt
    nsg = (nblk + 127) // 128

    nc = bacc.Bacc("TRN2", target_bir_lowering=False, debug=False)
    lg = nc.dram_tensor("logits", [rows, C], mybir.dt.float32, kind="ExternalInput").ap()
    s_d = nc.dram_tensor("s", [128, nsg * 512], mybir.dt.float32, kind="ExternalOutput").ap()
    mt_d = nc.dram_tensor("mt", [128, tt], mybir.dt.float32, kind="ExternalOutput").ap()

    # [p, t, (j c)] view: row t*1024 + p*8 + j; (j c) is 4KB-contiguous per (p,t)
    lg_t = (
        lg[0 : tfull * GR, :].rearrange("(t p j) c -> p t (j c)", p=128, j=G)
        if tfull
        else None
    )

    with tile.TileContext(nc) as tc, ExitStack() as ctx:
        singles = ctx.enter_context(tc.tile_pool(name="singles", bufs=1))
        xpool = ctx.enter_context(tc.tile_pool(name="x", bufs=5))
        epool = ctx.enter_context(tc.tile_pool(name="e", bufs=3 + M_DELAY))
        tpsum = ctx.enter_context(tc.tile_pool(name="tp", bufs=3, space="PSUM"))
        spsum = ctx.enter_context(tc.tile_pool(name="sp", bufs=1, space="PSUM"))

        ident = singles.tile([128, 128], mybir.dt.float32)
        make_identity(nc, ident[:])
        ident_bf = singles.tile([128, 128], mybir.dt.bfloat16)
        make_identity(nc, ident_bf[:])
        # sliding one-hot stationary: onehot[:, 127-k : 255-k] has its 1 at col k
        onehot = singles.tile([128, 255], mybir.dt.bfloat16)
        nc.vector.memset(onehot[:], 0.0)
        nc.vector.memset(onehot[:, 127:128], 1.0)

        mt_sb = singles.tile([128, tt], mybir.dt.float32)
        s_sb = singles.tile([128, nsg * 512], mybir.dt.float32)
        s_ps = [
            spsum.tile([128, 512], mybir.dt.float32, name=f"s_ps{i}")
            for i in range(nsg)
        ]
        s_count = [0] * nsg  # matmuls issued into each sum group
        s_total = [0] * nsg  # matmuls each group will receive
        for b in range(nblk):
            s_total[b // 128] += 1

        # chunk schedule: tail first (its memset off the drain path), 2-t-group
        # ramp-in, 8-t-group body, 2/2/2-t-group taper to shorten the drain
        chunks = []
        t0 = 0
        ramp = [1, 2]
        while t0 < tfull:
            left = tfull - t0
            if ramp:
                n = min(ramp.pop(0), left)
            elif left > chunk_t + 6:
                n = chunk_t
            elif left > 6:
                n = left - 6
            elif left > 4:
                n = left - 4
            elif left > 2:
                n = left - 2
            else:
                n = 1
            chunks.append([t0, n, False])
            t0 += n
        if tail:
            chunks.insert(1, [tfull, 0, True])

        pending_m = []  # (et_tile, global_t) awaiting their sum-matmuls

        def flush_m(limit):
            while len(pending_m) > limit:
                et, gt = pending_m.pop(0)
                for h in (0, 1):
                    b = 2 * gt + h
                    sg = b // 128
                    k = b % 128
                    nc.tensor.matmul(
                        s_ps[sg][:],
                        onehot[:, 127 - k : 255 - k],
                        et[:, h * 512 : (h + 1) * 512],
                        start=(s_count[sg] == 0),
                        stop=(s_count[sg] == s_total[sg] - 1),
                        skip_group_check=True,
                    )
                    s_count[sg] += 1

        for ci, (t0, nfull, has_tail) in enumerate(chunks):
            ntg = nfull + (1 if has_tail else 0)
            ncols = ntg * G
            x = xpool.tile([128, ncols, C], mybir.dt.float32)
            dma_eng = nc.scalar if ci < 2 else nc.sync
            for h0, h1 in ((0, nfull // 2), (nfull // 2, nfull)):
                if h1 > h0:
                    dma_eng.dma_start(
                        x[:, h0 * G : h1 * G, :].rearrange(
                            "p a c -> p (a c)"
                        ).rearrange("p (t b) -> p t b", b=G * C),
                        lg_t[:, t0 + h0 : t0 + h1, :],
                    )
            if has_tail:
                nc.vector.memset(x[:, nfull * G :, :], 0.0)
                tail_src = lg[tfull * GR : rows, :].rearrange("(p j) c -> p (j c)", j=G)
                nc.sync.dma_start(
                    x[0:tail_p, nfull * G :, :].rearrange("p a c -> p (a c)"), tail_src
                )

            # exact row max on DVE (the one 1x pass we keep)
            nc.vector.reduce_max(
                mt_sb[:, t0 * G : t0 * G + ncols], x[:],
                axis=mybir.AxisListType.X,
            )

            for lt in range(ntg):
                gt = t0 + lt  # global t-group id
                tp = tpsum.tile([128, 1024], mybir.dt.float32)
                for j in range(8):
                    nc.tensor.matmul(
                        tp[:, j * 128 : (j + 1) * 128],
                        x[:, lt * G + j, :],
                        ident[:],
                        is_transpose=True,
                        skip_group_check=True,
                    )
                et = epool.tile([128, 1024], mybir.dt.bfloat16)
                nc.scalar.activation(
                    et[:], tp[:], mybir.ActivationFunctionType.Exp
                )
                pending_m.append((et, gt))
                flush_m(M_DELAY)

            # stream this chunk's maxes out
            lo, hi = t0 * G, t0 * G + ncols
            nc.sync.dma_start(mt_d[:, lo:hi], mt_sb[:, lo:hi])

        flush_m(0)
        for sg in range(nsg):
            nc.vector.tensor_copy(
                s_sb[:, sg * 512 : (sg + 1) * 512], s_ps[sg][:]
            )
            nc.sync.dma_start(
                s_d[:, sg * 512 : (sg + 1) * 512],
                s_sb[:, sg * 512 : (sg + 1) * 512],
            )

    nc.compile()
    return nc


def _unpermute_mt(a_2d, rows):
    """Device mt [128, TT] -> per-row vector [rows].

    Column t*G+j, partition p <-> row t*G*128 + p*G + j.
    """
    tfull = rows // GR
    tail = rows - tfull * GR
    tail_p = tail // G
    out = np.empty(rows, a_2d.dtype)
    nmain = tfull * GR
    out[:nmain] = (
        a_2d[:, : tfull * G].reshape(128, tfull, G).transpose(1, 0, 2).reshape(-1)
    )
    if tail:
        out[nmain:] = a_2d[:tail_p, tfull * G :].reshape(-1)
    return out


def _unpermute_s(s_2d, rows):
    """Device s [128, NSG*512] -> per-row sum vector [rows].

    s_2d[k, S*512 + m] = sum for block B = S*128 + k, which covers row
    g*1024 + p*8 + j with g = B//2, h = B%2, j = 4h + m//128, p = m%128.
    """
    tfull = rows // GR
    tail = rows - tfull * GR
    nt = tfull + (1 if tail else 0)
    nblk = 2 * nt
    nsg = (nblk + 127) // 128
    blocks = (
        s_2d.reshape(128, nsg, 512).transpose(1, 0, 2).reshape(nsg * 128, 512)[:nblk]
    )
    # [B, m] -> [g, h, j', p] -> row-major (g, p, j=(h,j'))
    s_rows = blocks.reshape(nt, 2, 4, 128).transpose(0, 3, 1, 2)
    return s_rows.reshape(-1)[:rows].copy()


def _finish(conf, acc):
    """Mirror of the reference ECE finishing on host."""
    n = conf.shape[0]
    sorted_conf = np.sort(conf)
    q = np.linspace(0.0, float(n), NBINS + 1, dtype=np.float32)
    edges = np.interp(q, np.arange(n, dtype=np.float32), sorted_conf).astype(np.float32)
    idx = np.searchsorted(edges[1:-1], conf, side="left")
    valid = (conf > edges[0]) & (conf <= edges[-1])
    idx = np.where(valid, idx, NBINS)
    cnt = np.bincount(idx, minlength=NBINS + 1)[:NBINS].astype(np.float32)
    csum = np.bincount(idx, weights=conf.astype(np.float64), minlength=NBINS + 1)[
        :NBINS
    ].astype(np.float32)
    asum = np.bincount(idx, weights=acc.astype(np.float64), minlength=NBINS + 1)[
        :NBINS
    ].astype(np.float32)
    prop = cnt / np.float32(n)
    safe = np.maximum(cnt, 1.0)
    gap = np.abs(csum / safe - asum / safe)
    ece = np.sum(np.where(cnt > 0, gap * prop, 0.0), dtype=np.float32)
    return np.asarray(ece, dtype=np.float32).reshape(1)


def kernel(logits, labels, trace: bool = False):
    global LAST_RESULT
    logits = np.asarray(logits)
    labels = np.asarray(labels)
    assert logits.shape == (N, C), logits.shape

    if "nc" not in _CACHE:
        _CACHE["nc"] = _build(ROWS)
    nc = _CACHE["nc"]

    in_maps = [
        {"logits": np.ascontiguousarray(logits[i * ROWS : (i + 1) * ROWS], np.float32)}
        for i in range(NCORES)
    ]
    res = run_bass_kernel_spmd(nc, in_maps, core_ids=list(range(NCORES)), trace=trace)
    LAST_RESULT = res

    s = np.empty(N, np.float32)
    mt = np.empty(N, np.float32)
    for i in range(NCORES):
        s[i * ROWS : (i + 1) * ROWS] = _unpermute_s(res.results[i]["s"], ROWS)
        mt[i * ROWS : (i + 1) * ROWS] = _unpermute_mt(res.results[i]["mt"], ROWS)

    # mt = exact per-row max (f32); accuracy = logit at the label equals it
    xlab = logits[np.arange(N), labels.astype(np.int64)]
    acc = (xlab == mt).astype(np.float32)
    conf = (np.exp(mt) / s).astype(np.float32)
    return _finish(conf, acc)


# revision 17
# speedup vs baseline: 1.0520x; 1.0520x over previous
"""AdaptiveECE on 8 Trainium2 NeuronCores — v2 (PE-offloaded softmax sums).

Data-parallel over N=1,000,000 rows: each core streams its 125,000-row shard
of logits [N,128] through SBUF once (64MB/core, ~179us at the 358GB/s/core
HBM roofline) and reduces it to two per-row scalars:

  - mt[r] = max_c x[r,c]       exact f32 (VectorE segmented reduce_max, the
                               only 1x-rate DVE pass we keep: ~131us)
  - s[r]  = sum_c exp(x[r,c])  via TensorE+ScalarE instead of DVE:
      1. PE transposes each [128 rows, 128 cols] f32 block into PSUM
         (is_transpose matmul vs identity, 2 cyc/row: ~105us)
      2. ScalarE computes exp on the PSUM-resident transposed block, writing
         bf16 to SBUF (1 elem/cycle/lane @1.2GHz, dtype-free: ~123us)
      3. PE contracts the transposed exp over partitions (=columns) with a
         sliding one-hot stationary so each 512-row block's sums land on its
         own PSUM partition; 128 blocks accumulate into one PSUM bank
         (bf16 matmul 1 cyc/row: ~53us), then one cheap DVE copy evacuates
         65,536 row-sums at once.

  v1 ran both segmented reductions on DVE at its 1x tensor_reduce rate plus
  many small ScalarE accum ops (~226us busy on each), so compute — not the
  64MB stream — was the critical path (249us). v2 puts every engine below
  the DMA roofline: DVE ~135us, ACT ~125us, PE ~160us, DMA ~181us.

The host finishes with O(N) work as the sharding hint prescribes ("finish
ECE on one host"): conf = exp(mt)/s, accuracy = (logits[r, labels[r]] ==
mt[r]) — exact since mt is the bit-exact max — then the global sort,
equal-count bin edges, per-bin (count, conf_sum, acc_sum), and the ECE.

Layout: each partition line holds G=8 consecutive rows (4KB contiguous DMA
runs). mt column (t*G + j), partition p  <->  shard row t*G*128 + p*G + j.
Sums come out block-indexed: s_d[k, S*512 + m] = sum of row g*1024 + p*8 +
(4h + m//128) with p = m%128, where B = S*128 + k = 2g + h.
"""

import sys
import types
from contextlib import ExitStack

import numpy as np

import concourse.bass as bass
import concourse.tile as tile
from concourse import bacc, mybir
from concourse.bass_utils import run_bass_kernel_spmd
from concourse.masks import make_identity


def _ensure_ntff_hook():
    """bass_utils imports antenv.axon_hooks when tracing is requested; the
    agent image lacks that module. Recreate it (wired to the axon .so) so a
    stray BASS_TRACE=1 in the environment cannot crash the run."""
    try:
        import antenv.axon_hooks  # noqa: F401
        return
    except ImportError:
        pass
    try:
        import antenv
        import trn_agent_boot.trn_boot as tb

        mod = types.ModuleType("antenv.axon_hooks")
        holder = [None]
        mod.set_axon_ntff_profile_hook = lambda h: holder.__setitem__(0, h)
        mod.get_axon_ntff_profile_hook = lambda: holder[0]
        sys.modules["antenv.axon_hooks"] = mod
        antenv.axon_hooks = mod
        try:
            mod.set_axon_ntff_profile_hook(
                tb._ntff_profile_via_ctypes("/opt/axon/libaxon_pjrt.so")
            )
        except Exception:
            pass
    except Exception:
        pass


_ensure_ntff_hook()

N = 1_000_000
C = 128
NBINS = 15
NCORES = 8
ROWS = N // NCORES  # 125_000 per core
G = 8  # rows per partition line (4KB contiguous DMA runs)
GR = G * 128  # rows per t-group
TFULL = ROWS // GR  # 122 full t-groups
TAIL = ROWS - TFULL * GR  # 72 leftover rows
TAIL_P = TAIL // G  # 9 tail partitions
NT = TFULL + 1  # t-groups incl. zero-padded tail
NBLK = 2 * NT  # 512-row sum blocks
NSG = (NBLK + 127) // 128  # PSUM sum groups (2)
CHUNK_T = 8  # t-groups per DMA chunk
M_DELAY = 2  # t-groups between exp and its sum-matmuls (keeps PE unstalled)
USE_FP32R = False  # fp32r transposes: 1.5 cyc/row vs fp32's 2 on the PE

_CACHE: dict = {}
LAST_RESULT = None  # BassKernelResults of the most recent device run


def _build(rows: int, chunk_t: int = CHUNK_T):
    tfull = rows // GR
    tail = rows - tfull * GR
    tail_p = tail // G
    assert tail % G == 0, (rows, tail)
    nt = tfull + (1 if tail else 0)
    tt = nt * G  # mt output columns
    nblk = 2 * nt
    nsg = (nblk + 127) // 128

    nc = bacc.Bacc("TRN2", target_bir_lowering=False, debug=False)
    lg = nc.dram_tensor("logits", [rows, C], mybir.dt.float32, kind="ExternalInput").ap()
    s_d = nc.dram_tensor("s", [128, nsg * 512], mybir.dt.float32, kind="ExternalOutput").ap()
    mt_d = nc.dram_tensor("mt", [128, tt], mybir.dt.float32, kind="ExternalOutput").ap()

    # [p, t, (j c)] view: row t*1024 + p*8 + j; (j c) is 4KB-contiguous per (p,t)
    lg_t = (
        lg[0 : tfull * GR, :].rearrange("(t p j) c -> p t (j c)", p=128, j=G)
        if tfull
        else None
    )

    with tile.TileContext(nc) as tc, ExitStack() as ctx:
        singles = ctx.enter_context(tc.tile_pool(name="singles", bufs=1))
        xpool = ctx.enter_context(tc.tile_pool(name="x", bufs=5))
        epool = ctx.enter_context(tc.tile_pool(name="e", bufs=3 + M_DELAY))
        tpsum = ctx.enter_context(tc.tile_pool(name="tp", bufs=3, space="PSUM"))
        spsum = ctx.enter_context(tc.tile_pool(name="sp", bufs=1, space="PSUM"))

        ident = singles.tile([128, 128], mybir.dt.float32)
        make_identity(nc, ident[:])
        ident_bf = singles.tile([128, 128], mybir.dt.bfloat16)
        make_identity(nc, ident_bf[:])
        # sliding one-hot stationary: onehot[:, 127-k : 255-k] has its 1 at col k
        onehot = singles.tile([128, 255], mybir.dt.bfloat16)
        nc.vector.memset(onehot[:], 0.0)
        nc.vector.memset(onehot[:, 127:128], 1.0)

        mt_sb = singles.tile([128, tt], mybir.dt.float32)
        s_sb = singles.tile([128, nsg * 512], mybir.dt.float32)
        s_ps = [
            spsum.tile([128, 512], mybir.dt.float32, name=f"s_ps{i}")
            for i in range(nsg)
        ]
        s_count = [0] * nsg  # matmuls issued into each sum group
        s_total = [0] * nsg  # matmuls each group will receive
        for b in range(nblk):
            s_total[b // 128] += 1

        # chunk schedule: tail first (its memset off the drain path), 2-t-group
        # ramp-in, 8-t-group body, 2/2/2-t-group taper to shorten the drain
        chunks = []
        t0 = 0
        ramp = [1, 2]
        while t0 < tfull:
            left = tfull - t0
            if ramp:
                n = min(ramp.pop(0), left)
            elif left > chunk_t + 6:
                n = chunk_t
            elif left > 6:
                n = left - 6
            elif left > 4:
                n = left - 4
            elif left > 2:
                n = left - 2
            else:
                n = left
            chunks.append([t0, n, False])
            t0 += n
        if tail:
            chunks.insert(1, [tfull, 0, True])

        pending_m = []  # (et_tile, global_t) awaiting their sum-matmuls

        def flush_m(limit):
            while len(pending_m) > limit:
                et, gt = pending_m.pop(0)
                for h in (0, 1):
                    b = 2 * gt + h
                    sg = b // 128
                    k = b % 128
                    nc.tensor.matmul(
                        s_ps[sg][:],
                        onehot[:, 127 - k : 255 - k],
                        et[:, h * 512 : (h + 1) * 512],
                        start=(s_count[sg] == 0),
                        stop=(s_count[sg] == s_total[sg] - 1),
                        skip_group_check=True,
                    )
                    s_count[sg] += 1

        for t0, nfull, has_tail in chunks:
            ntg = nfull + (1 if has_tail else 0)
            ncols = ntg * G
            x = xpool.tile([128, ncols, C], mybir.dt.float32)
            for h0, h1 in ((0, nfull // 2), (nfull // 2, nfull)):
                if h1 > h0:
                    nc.sync.dma_start(
                        x[:, h0 * G : h1 * G, :].rearrange(
                            "p a c -> p (a c)"
                        ).rearrange("p (t b) -> p t b", b=G * C),
                        lg_t[:, t0 + h0 : t0 + h1, :],
                    )
            if has_tail:
                nc.vector.memset(x[:, nfull * G :, :], 0.0)
                tail_src = lg[tfull * GR : rows, :].rearrange("(p j) c -> p (j c)", j=G)
                nc.sync.dma_start(
                    x[0:tail_p, nfull * G :, :].rearrange("p a c -> p (a c)"), tail_src
                )

            # exact row max on DVE (the one 1x pass we keep)
            nc.vector.reduce_max(
                mt_sb[:, t0 * G : t0 * G + ncols], x[:],
                axis=mybir.AxisListType.X,
            )

            for lt in range(ntg):
                gt = t0 + lt  # global t-group id
                tp = tpsum.tile([128, 1024], mybir.dt.float32)
                for j in range(8):
                    nc.tensor.matmul(
                        tp[:, j * 128 : (j + 1) * 128],
                        x[:, lt * G + j, :],
                        ident[:],
                        is_transpose=True,
                        skip_group_check=True,
                    )
                et = epool.tile([128, 1024], mybir.dt.bfloat16)
                nc.scalar.activation(
                    et[:], tp[:], mybir.ActivationFunctionType.Exp
                )
                pending_m.append((et, gt))
                flush_m(M_DELAY)

            # stream this chunk's maxes out
            lo, hi = t0 * G, t0 * G + ncols
            nc.sync.dma_start(mt_d[:, lo:hi], mt_sb[:, lo:hi])

        flush_m(0)
        for sg in range(nsg):
            nc.vector.tensor_copy(
                s_sb[:, sg * 512 : (sg + 1) * 512], s_ps[sg][:]
            )
            nc.sync.dma_start(
                s_d[:, sg * 512 : (sg + 1) * 512],
                s_sb[:, sg * 512 : (sg + 1) * 512],
            )

    nc.compile()
    return nc


def _unpermute_mt(a_2d, rows):
    """Device mt [128, TT] -> per-row vector [rows].

    Column t*G+j, partition p <-> row t*G*128 + p*G + j.
    """
    tfull = rows // GR
    tail = rows - tfull * GR
    tail_p = tail // G
    out = np.empty(rows, a_2d.dtype)
    nmain = tfull * GR
    out[:nmain] = (
        a_2d[:, : tfull * G].reshape(128, tfull, G).transpose(1, 0, 2).reshape(-1)
    )
    if tail:
        out[nmain:] = a_2d[:tail_p, tfull * G :].reshape(-1)
    return out


def _unpermute_s(s_2d, rows):
    """Device s [128, NSG*512] -> per-row sum vector [rows].

    s_2d[k, S*512 + m] = sum for block B = S*128 + k, which covers row
    g*1024 + p*8 + j with g = B//2, h = B%2, j = 4h + m//128, p = m%128.
    """
    tfull = rows // GR
    tail = rows - tfull * GR
    nt = tfull + (1 if tail else 0)
    nblk = 2 * nt
    nsg = (nblk + 127) // 128
    blocks = (
        s_2d.reshape(128, nsg, 512).transpose(1, 0, 2).reshape(nsg * 128, 512)[:nblk]
    )
    # [B, m] -> [g, h, j', p] -> row-major (g, p, j=(h,j'))
    s_rows = blocks.reshape(nt, 2, 4, 128).transpose(0, 3, 1, 2)
    return s_rows.reshape(-1)[:rows].copy()


def _finish(conf, acc):
    """Mirror of the reference ECE finishing on host."""
    n = conf.shape[0]
    sorted_conf = np.sort(conf)
    q = np.linspace(0.0, float(n), NBINS + 1, dtype=np.float32)
    edges = np.interp(q, np.arange(n, dtype=np.float32), sorted_conf).astype(np.float32)
    idx = np.searchsorted(edges[1:-1], conf, side="left")
    valid = (conf > edges[0]) & (conf <= edges[-1])
    idx = np.where(valid, idx, NBINS)
    cnt = np.bincount(idx, minlength=NBINS + 1)[:NBINS].astype(np.float32)
    csum = np.bincount(idx, weights=conf.astype(np.float64), minlength=NBINS + 1)[
        :NBINS
    ].astype(np.float32)
    asum = np.bincount(idx, weights=acc.astype(np.float64), minlength=NBINS + 1)[
        :NBINS
    ].astype(np.float32)
    prop = cnt / np.float32(n)
    safe = np.maximum(cnt, 1.0)
    gap = np.abs(csum / safe - asum / safe)
    ece = np.sum(np.where(cnt > 0, gap * prop, 0.0), dtype=np.float32)
    return np.asarray(ece, dtype=np.float32).reshape(1)


def kernel(logits, labels, trace: bool = False):
    global LAST_RESULT
    logits = np.asarray(logits)
    labels = np.asarray(labels)
    assert logits.shape == (N, C), logits.shape

    if "nc" not in _CACHE:
        _CACHE["nc"] = _build(ROWS)
    nc = _CACHE["nc"]

    in_maps = [
        {"logits": np.ascontiguousarray(logits[i * ROWS : (i + 1) * ROWS], np.float32)}
        for i in range(NCORES)
    ]
    res = run_bass_kernel_spmd(nc, in_maps, core_ids=list(range(NCORES)), trace=trace)
    LAST_RESULT = res

    s = np.empty(N, np.float32)
    mt = np.empty(N, np.float32)
    for i in range(NCORES):
        s[i * ROWS : (i + 1) * ROWS] = _unpermute_s(res.results[i]["s"], ROWS)
        mt[i * ROWS : (i + 1) * ROWS] = _unpermute_mt(res.results[i]["mt"], ROWS)

    # mt = exact per-row max (f32); accuracy = logit at the label equals it
    xlab = logits[np.arange(N), labels.astype(np.int64)]
    acc = (xlab == mt).astype(np.float32)
    conf = (np.exp(mt) / s).astype(np.float32)
    return _finish(conf, acc)


# revision 18
# speedup vs baseline: 1.0750x; 1.0218x over previous
"""AdaptiveECE on 8 Trainium2 NeuronCores — v2 (PE-offloaded softmax sums).

Data-parallel over N=1,000,000 rows: each core streams its 125,000-row shard
of logits [N,128] through SBUF once (64MB/core, ~179us at the 358GB/s/core
HBM roofline) and reduces it to two per-row scalars:

  - mt[r] = max_c x[r,c]       exact f32 (VectorE segmented reduce_max, the
                               only 1x-rate DVE pass we keep: ~131us)
  - s[r]  = sum_c exp(x[r,c])  via TensorE+ScalarE instead of DVE:
      1. PE transposes each [128 rows, 128 cols] f32 block into PSUM
         (is_transpose matmul vs identity, 2 cyc/row: ~105us)
      2. ScalarE computes exp on the PSUM-resident transposed block, writing
         bf16 to SBUF (1 elem/cycle/lane @1.2GHz, dtype-free: ~123us)
      3. PE contracts the transposed exp over partitions (=columns) with a
         sliding one-hot stationary so each 512-row block's sums land on its
         own PSUM partition; 128 blocks accumulate into one PSUM bank
         (bf16 matmul 1 cyc/row: ~53us), then one cheap DVE copy evacuates
         65,536 row-sums at once.

  v1 ran both segmented reductions on DVE at its 1x tensor_reduce rate plus
  many small ScalarE accum ops (~226us busy on each), so compute — not the
  64MB stream — was the critical path (249us). v2 makes the DMA stream the
  pacer: measured engine-busy DVE ~147us, ACT ~142us, PE ~170-176us, DMA
  ~186us; HW exec ~206-211us (run-to-run clock jitter a few us). PSUM is
  fully allocated: 3 transpose-staging tiles x 2 banks + 2 sum banks —
  fewer staging tiles starves the PE behind ScalarE's drain.

The host finishes with O(N) work as the sharding hint prescribes ("finish
ECE on one host"): conf = exp(mt)/s, accuracy = (logits[r, labels[r]] ==
mt[r]) — exact since mt is the bit-exact max — then the global sort,
equal-count bin edges, per-bin (count, conf_sum, acc_sum), and the ECE.

Layout: each partition line holds G=8 consecutive rows (4KB contiguous DMA
runs). mt column (t*G + j), partition p  <->  shard row t*G*128 + p*G + j.
Sums come out block-indexed: s_d[k, S*512 + m] = sum of row g*1024 + p*8 +
(4h + m//128) with p = m%128, where B = S*128 + k = 2g + h.
"""

import sys
import types
from contextlib import ExitStack

import numpy as np

import concourse.bass as bass
import concourse.tile as tile
from concourse import bacc, mybir
from concourse.bass_utils import run_bass_kernel_spmd
from concourse.masks import make_identity


def _ensure_ntff_hook():
    """bass_utils imports antenv.axon_hooks when tracing is requested; the
    agent image lacks that module. Recreate it (wired to the axon .so) so a
    stray BASS_TRACE=1 in the environment cannot crash the run."""
    try:
        import antenv.axon_hooks  # noqa: F401
        return
    except ImportError:
        pass
    try:
        import antenv
        import trn_agent_boot.trn_boot as tb

        mod = types.ModuleType("antenv.axon_hooks")
        holder = [None]
        mod.set_axon_ntff_profile_hook = lambda h: holder.__setitem__(0, h)
        mod.get_axon_ntff_profile_hook = lambda: holder[0]
        sys.modules["antenv.axon_hooks"] = mod
        antenv.axon_hooks = mod
        try:
            mod.set_axon_ntff_profile_hook(
                tb._ntff_profile_via_ctypes("/opt/axon/libaxon_pjrt.so")
            )
        except Exception:
            pass
    except Exception:
        pass


_ensure_ntff_hook()

N = 1_000_000
C = 128
NBINS = 15
NCORES = 8
ROWS = N // NCORES  # 125_000 per core
G = 8  # rows per partition line (4KB contiguous DMA runs)
GR = G * 128  # rows per t-group
TFULL = ROWS // GR  # 122 full t-groups
TAIL = ROWS - TFULL * GR  # 72 leftover rows
TAIL_P = TAIL // G  # 9 tail partitions
NT = TFULL + 1  # t-groups incl. zero-padded tail
NBLK = 2 * NT  # 512-row sum blocks
NSG = (NBLK + 127) // 128  # PSUM sum groups (2)
CHUNK_T = 8  # t-groups per DMA chunk
M_DELAY = 2  # t-groups between exp and its sum-matmuls (keeps PE unstalled)

_CACHE: dict = {}
LAST_RESULT = None  # BassKernelResults of the most recent device run


def _build(rows: int, chunk_t: int = CHUNK_T):
    tfull = rows // GR
    tail = rows - tfull * GR
    tail_p = tail // G
    assert tail % G == 0, (rows, tail)
    nt = tfull + (1 if tail else 0)
    tt = nt * G  # mt output columns
    nblk = 2 * nt
    nsg = (nblk + 127) // 128

    nc = bacc.Bacc("TRN2", target_bir_lowering=False, debug=False)
    lg = nc.dram_tensor("logits", [rows, C], mybir.dt.float32, kind="ExternalInput").ap()
    s_d = nc.dram_tensor("s", [128, nsg * 512], mybir.dt.float32, kind="ExternalOutput").ap()
    mt_d = nc.dram_tensor("mt", [128, tt], mybir.dt.float32, kind="ExternalOutput").ap()

    # [p, t, (j c)] view: row t*1024 + p*8 + j; (j c) is 4KB-contiguous per (p,t)
    lg_t = (
        lg[0 : tfull * GR, :].rearrange("(t p j) c -> p t (j c)", p=128, j=G)
        if tfull
        else None
    )

    with tile.TileContext(nc) as tc, ExitStack() as ctx:
        singles = ctx.enter_context(tc.tile_pool(name="singles", bufs=1))
        xpool = ctx.enter_context(tc.tile_pool(name="x", bufs=5))
        epool = ctx.enter_context(tc.tile_pool(name="e", bufs=3 + M_DELAY))
        tpsum = ctx.enter_context(tc.tile_pool(name="tp", bufs=3, space="PSUM"))
        spsum = ctx.enter_context(tc.tile_pool(name="sp", bufs=1, space="PSUM"))

        ident = singles.tile([128, 128], mybir.dt.float32)
        make_identity(nc, ident[:])
        # sliding one-hot stationary: onehot[:, 127-k : 255-k] has its 1 at col k
        onehot = singles.tile([128, 255], mybir.dt.bfloat16)
        nc.vector.memset(onehot[:], 0.0)
        nc.vector.memset(onehot[:, 127:128], 1.0)

        mt_sb = singles.tile([128, tt], mybir.dt.float32)
        s_sb = singles.tile([128, nsg * 512], mybir.dt.float32)
        s_ps = [
            spsum.tile([128, 512], mybir.dt.float32, name=f"s_ps{i}")
            for i in range(nsg)
        ]
        s_count = [0] * nsg  # matmuls issued into each sum group
        s_total = [0] * nsg  # matmuls each group will receive
        for b in range(nblk):
            s_total[b // 128] += 1

        # chunk schedule: tail first (its memset off the drain path), 2-t-group
        # ramp-in, 8-t-group body, 2/2/2-t-group taper to shorten the drain
        chunks = []
        t0 = 0
        ramp = [1, 2]
        while t0 < tfull:
            left = tfull - t0
            if ramp:
                n = min(ramp.pop(0), left)
            elif left > chunk_t + 6:
                n = chunk_t
            elif left > 6:
                n = left - 6
            elif left > 4:
                n = left - 4
            elif left > 2:
                n = left - 2
            else:
                n = left
            chunks.append([t0, n, False])
            t0 += n
        if tail:
            chunks.insert(1, [tfull, 0, True])

        pending_m = []  # (et_tile, global_t) awaiting their sum-matmuls

        def flush_m(limit):
            while len(pending_m) > limit:
                et, gt = pending_m.pop(0)
                for h in (0, 1):
                    b = 2 * gt + h
                    sg = b // 128
                    k = b % 128
                    nc.tensor.matmul(
                        s_ps[sg][:],
                        onehot[:, 127 - k : 255 - k],
                        et[:, h * 512 : (h + 1) * 512],
                        start=(s_count[sg] == 0),
                        stop=(s_count[sg] == s_total[sg] - 1),
                        skip_group_check=True,
                    )
                    s_count[sg] += 1

        for t0, nfull, has_tail in chunks:
            ntg = nfull + (1 if has_tail else 0)
            ncols = ntg * G
            x = xpool.tile([128, ncols, C], mybir.dt.float32)
            for h0, h1 in ((0, nfull // 2), (nfull // 2, nfull)):
                if h1 > h0:
                    nc.sync.dma_start(
                        x[:, h0 * G : h1 * G, :].rearrange(
                            "p a c -> p (a c)"
                        ).rearrange("p (t b) -> p t b", b=G * C),
                        lg_t[:, t0 + h0 : t0 + h1, :],
                    )
            if has_tail:
                nc.vector.memset(x[:, nfull * G :, :], 0.0)
                tail_src = lg[tfull * GR : rows, :].rearrange("(p j) c -> p (j c)", j=G)
                nc.sync.dma_start(
                    x[0:tail_p, nfull * G :, :].rearrange("p a c -> p (a c)"), tail_src
                )

            # exact row max on DVE (the one 1x pass we keep)
            nc.vector.reduce_max(
                mt_sb[:, t0 * G : t0 * G + ncols], x[:],
                axis=mybir.AxisListType.X,
            )

            for lt in range(ntg):
                gt = t0 + lt  # global t-group id
                tp = tpsum.tile([128, 1024], mybir.dt.float32)
                for j in range(8):
                    nc.tensor.matmul(
                        tp[:, j * 128 : (j + 1) * 128],
                        x[:, lt * G + j, :],
                        ident[:],
                        is_transpose=True,
                        skip_group_check=True,
                    )
                et = epool.tile([128, 1024], mybir.dt.bfloat16)
                nc.scalar.activation(
                    et[:], tp[:], mybir.ActivationFunctionType.Exp
                )
                pending_m.append((et, gt))
                flush_m(M_DELAY)

            # stream this chunk's maxes out
            lo, hi = t0 * G, t0 * G + ncols
            nc.sync.dma_start(mt_d[:, lo:hi], mt_sb[:, lo:hi])

        flush_m(0)
        for sg in range(nsg):
            nc.vector.tensor_copy(
                s_sb[:, sg * 512 : (sg + 1) * 512], s_ps[sg][:]
            )
            nc.sync.dma_start(
                s_d[:, sg * 512 : (sg + 1) * 512],
                s_sb[:, sg * 512 : (sg + 1) * 512],
            )

    nc.compile()
    return nc


def _unpermute_mt(a_2d, rows):
    """Device mt [128, TT] -> per-row vector [rows].

    Column t*G+j, partition p <-> row t*G*128 + p*G + j.
    """
    tfull = rows // GR
    tail = rows - tfull * GR
    tail_p = tail // G
    out = np.empty(rows, a_2d.dtype)
    nmain = tfull * GR
    out[:nmain] = (
        a_2d[:, : tfull * G].reshape(128, tfull, G).transpose(1, 0, 2).reshape(-1)
    )
    if tail:
        out[nmain:] = a_2d[:tail_p, tfull * G :].reshape(-1)
    return out


def _unpermute_s(s_2d, rows):
    """Device s [128, NSG*512] -> per-row sum vector [rows].

    s_2d[k, S*512 + m] = sum for block B = S*128 + k, which covers row
    g*1024 + p*8 + j with g = B//2, h = B%2, j = 4h + m//128, p = m%128.
    """
    tfull = rows // GR
    tail = rows - tfull * GR
    nt = tfull + (1 if tail else 0)
    nblk = 2 * nt
    nsg = (nblk + 127) // 128
    blocks = (
        s_2d.reshape(128, nsg, 512).transpose(1, 0, 2).reshape(nsg * 128, 512)[:nblk]
    )
    # [B, m] -> [g, h, j', p] -> row-major (g, p, j=(h,j'))
    s_rows = blocks.reshape(nt, 2, 4, 128).transpose(0, 3, 1, 2)
    return s_rows.reshape(-1)[:rows].copy()


def _finish(conf, acc):
    """Mirror of the reference ECE finishing on host."""
    n = conf.shape[0]
    sorted_conf = np.sort(conf)
    q = np.linspace(0.0, float(n), NBINS + 1, dtype=np.float32)
    edges = np.interp(q, np.arange(n, dtype=np.float32), sorted_conf).astype(np.float32)
    idx = np.searchsorted(edges[1:-1], conf, side="left")
    valid = (conf > edges[0]) & (conf <= edges[-1])
    idx = np.where(valid, idx, NBINS)
    cnt = np.bincount(idx, minlength=NBINS + 1)[:NBINS].astype(np.float32)
    csum = np.bincount(idx, weights=conf.astype(np.float64), minlength=NBINS + 1)[
        :NBINS
    ].astype(np.float32)
    asum = np.bincount(idx, weights=acc.astype(np.float64), minlength=NBINS + 1)[
        :NBINS
    ].astype(np.float32)
    prop = cnt / np.float32(n)
    safe = np.maximum(cnt, 1.0)
    gap = np.abs(csum / safe - asum / safe)
    ece = np.sum(np.where(cnt > 0, gap * prop, 0.0), dtype=np.float32)
    return np.asarray(ece, dtype=np.float32).reshape(1)


def kernel(logits, labels, trace: bool = False):
    global LAST_RESULT
    logits = np.asarray(logits)
    labels = np.asarray(labels)
    assert logits.shape == (N, C), logits.shape

    if "nc" not in _CACHE:
        _CACHE["nc"] = _build(ROWS)
    nc = _CACHE["nc"]

    in_maps = [
        {"logits": np.ascontiguousarray(logits[i * ROWS : (i + 1) * ROWS], np.float32)}
        for i in range(NCORES)
    ]
    res = run_bass_kernel_spmd(nc, in_maps, core_ids=list(range(NCORES)), trace=trace)
    LAST_RESULT = res

    s = np.empty(N, np.float32)
    mt = np.empty(N, np.float32)
    for i in range(NCORES):
        s[i * ROWS : (i + 1) * ROWS] = _unpermute_s(res.results[i]["s"], ROWS)
        mt[i * ROWS : (i + 1) * ROWS] = _unpermute_mt(res.results[i]["mt"], ROWS)

    # mt = exact per-row max (f32); accuracy = logit at the label equals it
    xlab = logits[np.arange(N), labels.astype(np.int64)]
    acc = (xlab == mt).astype(np.float32)
    conf = (np.exp(mt) / s).astype(np.float32)
    return _finish(conf, acc)
